# revision 23
# baseline (speedup 1.0000x reference)
"""Trainium2 Bass kernel for nn_DigitCapsv2 (capsule routing over 3 modalities).

Mathematical structure exploited:
  In the reference, b1/b2/b3 start at zero, so the softmax coefficients c are
  constant across the 768-row capsule dimension. Hence every row of
  s = c1@tv + c2@ta + c3@tva is identical, and by induction a=[B,768,D] stays
  rank-1 in the capsule dim through all routing iterations. Per batch b the
  whole module reduces to vector math over the stacked modality matrix
  T = [tv;ta;tva] in R^{150 x 3072}:

     gamma_k[s] = exp(beta_k[s]-m[s]) / (768 * sum_k exp(beta_k[s]-m[s]))
     sigma = T^T gamma            (never materialized during routing)
     u     = T sigma = G gamma,   G = T T^T  (150x150 Gram, computed once)
     sq    = gamma^T u,  scale = sq/(1+sq)/(sqrt(sq)+eps)
     beta += scale * u
     out[b] = (Wsum_b * scale3) outer sigma3,  Wsum_b = W[b].sum(-1)

  G is computed on the tensor engine from a d-on-partitions packed layout;
  routing iterations are tiny 150-element ops; the final projection is
  out = C^T @ T_nat with rank-1 C = (gamma3*scale3) x Wsum on the tensor
  engine. No inter-core communication: batches are data-parallel.

Sharding: 10 batches over 8 cores, 2 slots per core (cores 0 and 2 take two
real batches; other cores pad slot 1 with zeros, discarded on gather).
"""

import numpy as np
from contextlib import ExitStack

import concourse.bass as bass
import concourse.bacc as bacc
import concourse.tile as tile
import concourse.mybir as mybir
from concourse.bass_utils import run_bass_kernel_spmd

F32 = mybir.dt.float32

B, S, D = 10, 50, 3072
R = 3 * S            # 150 stacked modality rows
NJ = D // 128        # 24 d-tiles
RA, RB = 128, R - 128  # partition split of the 150 rows
NSLOT = 2
NCHUNK = D // 512    # 6 output free-dim chunks
EPS = 1e-8
GAMMA1 = 1.0 / 2304.0

# core -> (slot0 batch, slot1 batch or None); two-batch cores on different
# HBM domains (pairs (0,1),(2,3),...)
ASSIGN = {0: (0, 1), 1: (4, None), 2: (2, 3), 3: (5, None),
          4: (6, None), 5: (7, None), 6: (8, None), 7: (9, None)}
N_CORES = 8

_PROGRAM_CACHE = {}


def _softmax_rows(nc, pool, beta, tag):
    """beta [1,150] -> gamma [1,150]; joint softmax over the 3 modality blocks.

    gamma_k[s] = exp(beta_k[s]-m[s]) / (768*sum_k exp(beta_k[s]-m[s]))
    """
    m = pool.tile([1, S], F32, tag=f"m_{tag}", name=f"m_{tag}")
    nc.vector.tensor_max(m[:], beta[:, 0:S], beta[:, S:2 * S])
    nc.vector.tensor_max(m[:], m[:], beta[:, 2 * S:3 * S])
    epre = pool.tile([1, R], F32, tag=f"epre_{tag}", name=f"epre_{tag}")
    for k in range(3):
        nc.vector.tensor_sub(epre[:, k * S:(k + 1) * S], beta[:, k * S:(k + 1) * S], m[:])
    e = pool.tile([1, R], F32, tag=f"e_{tag}", name=f"e_{tag}")
    nc.scalar.activation(e[:], epre[:], mybir.ActivationFunctionType.Exp)
    den = pool.tile([1, S], F32, tag=f"den_{tag}", name=f"den_{tag}")
    nc.vector.tensor_add(den[:], e[:, 0:S], e[:, S:2 * S])
    nc.vector.tensor_add(den[:], den[:], e[:, 2 * S:3 * S])
    nc.vector.tensor_scalar_mul(den[:], den[:], 768.0)
    dinv = pool.tile([1, S], F32, tag=f"dinv_{tag}", name=f"dinv_{tag}")
    nc.vector.reciprocal(dinv[:], den[:])
    gamma = pool.tile([1, R], F32, tag=f"gamma_{tag}", name=f"gamma_{tag}")
    for k in range(3):
        nc.vector.tensor_mul(gamma[:, k * S:(k + 1) * S], e[:, k * S:(k + 1) * S], dinv[:])
    return gamma


def _scale_chain(nc, pool, sq, tag):
    """sq [1,1] -> scale = sq/(1+sq)/(sqrt(sq)+EPS) as [1,1]."""
    onep = pool.tile([1, 1], F32, tag=f"onep_{tag}", name=f"onep_{tag}")
    nc.vector.tensor_scalar_add(onep[:], sq[:], 1.0)
    opinv = pool.tile([1, 1], F32, tag=f"opinv_{tag}", name=f"opinv_{tag}")
    nc.vector.reciprocal(opinv[:], onep[:])
    # sqrt via exp(0.5*ln(x)): keeps ACT on the ln/exp table set -- the
    # Sqrt function lives in a different act-table set and every switch
    # costs a ~1.3us table load. 1e-30 bias guards ln(0) on zero-padded slots.
    lnb = pool.tile([1, 1], F32, tag="lnbias", name="lnbias")
    nc.vector.memset(lnb[:], 1e-30)
    lnv = pool.tile([1, 1], F32, tag=f"lnv_{tag}", name=f"lnv_{tag}")
    nc.scalar.activation(lnv[:], sq[:], mybir.ActivationFunctionType.Ln, bias=lnb[0:1, 0:1])
    rt = pool.tile([1, 1], F32, tag=f"rt_{tag}", name=f"rt_{tag}")
    nc.scalar.activation(rt[:], lnv[:], mybir.ActivationFunctionType.Exp, scale=0.5)
    nc.vector.tensor_scalar_add(rt[:], rt[:], EPS)
    rtinv = pool.tile([1, 1], F32, tag=f"rtinv_{tag}", name=f"rtinv_{tag}")
    nc.vector.reciprocal(rtinv[:], rt[:])
    scale = pool.tile([1, 1], F32, tag=f"scale_{tag}", name=f"scale_{tag}")
    nc.vector.tensor_mul(scale[:], sq[:], opinv[:])
    nc.vector.tensor_mul(scale[:], scale[:], rtinv[:])
    return scale


def _transpose_row(nc, pool, psum, row, n_a, n_b, ones, tag):
    """row [1, n_a+n_b] -> (col_a [n_a,1], col_b [n_b,1]) SBUF columns via PE."""
    tp = psum.tile([n_a, 2], F32, tag="t", name="tps", bufs=2)
    nc.tensor.transpose(tp[0:n_a, 0:1], row[:, 0:n_a], ones[0:1, 0:1])
    nc.tensor.transpose(tp[0:n_b, 1:2], row[:, n_a:n_a + n_b], ones[0:1, 0:1])
    col_a = pool.tile([n_a, 1], F32, tag=f"col_{tag}", name=f"col_{tag}")
    nc.scalar.copy(col_a[:], tp[0:n_a, 0:1])
    col_b = pool.tile([n_b, 1], F32, tag=f"col_{tag}b", name=f"col_{tag}b")
    nc.scalar.copy(col_b[:], tp[0:n_b, 1:2])
    return col_a, col_b


def _patch_act_tables():
    """Make exp and ln resolve to the single act-table set that holds both
    (natural_log_exp_and_others), so the whole kernel needs ONE table load.
    Set ids (dict order) are preserved; we only hide exp/ln from other sets."""
    import concourse.bacc as _bacc
    import concourse.hw_specs as _hw
    if getattr(_patch_act_tables, "done", False):
        return
    orig = _hw.get_activation_tables

    def patched(module_arch):
        tables = dict(orig(module_arch))
        exp = mybir.ActivationFunctionType.Exp
        ln = mybir.ActivationFunctionType.Ln
        for name, funcs in tables.items():
            if name != "natural_log_exp_and_others":
                tables[name] = funcs - {exp, ln}
        return tables

    _hw.get_activation_tables = patched
    if hasattr(_bacc, "get_activation_tables"):
        _bacc.get_activation_tables = patched
    _patch_act_tables.done = True


def build_program():
    _patch_act_tables()
    nc = bacc.Bacc(None, target_bir_lowering=False)

    tpack = nc.dram_tensor("tpack", [NSLOT, 128, NJ * R], F32, kind="ExternalInput")
    tnat = nc.dram_tensor("tnat", [R, D], F32, kind="ExternalInput")       # slot0 only
    wrep = nc.dram_tensor("wrep", [R, S], F32, kind="ExternalInput")       # slot0 Wsum rep
    w128 = nc.dram_tensor("w128", [128, S], F32, kind="ExternalInput")     # slot1 Wsum rep
    ident = nc.dram_tensor("ident", [128, 128], F32, kind="ExternalInput")
    out0 = nc.dram_tensor("out0", [S, D], F32, kind="ExternalOutput")
    out1 = nc.dram_tensor("out1", [128, NJ * S], F32, kind="ExternalOutput")

    with ExitStack() as ctx:
        tc = ctx.enter_context(tile.TileContext(nc))
        main = ctx.enter_context(tc.tile_pool(name="main", bufs=1))
        small = ctx.enter_context(tc.tile_pool(name="small", bufs=1))
        gps = ctx.enter_context(tc.tile_pool(name="gps", bufs=2, space=bass.MemorySpace.PSUM))
        ups = ctx.enter_context(tc.tile_pool(name="ups", bufs=1, space=bass.MemorySpace.PSUM))
        ops = ctx.enter_context(tc.tile_pool(name="ops", bufs=1, space=bass.MemorySpace.PSUM))

        # constants
        ones = small.tile([1, 128], F32, tag="ones", name="ones")
        nc.vector.memset(ones[:], 1.0)
        ccol_a = small.tile([RA, 1], F32, tag="ccol_a", name="ccol_a")
        nc.vector.memset(ccol_a[:], GAMMA1)
        ccol_b = small.tile([RB, 1], F32, tag="ccol_b", name="ccol_b")
        nc.vector.memset(ccol_b[:], GAMMA1)

        # loads: Gram inputs first so PE can start ASAP; T_nat last (needed
        # only by slot0's final projection)
        TP = {}
        for s in range(NSLOT):
            TP[s] = main.tile([128, NJ * R], F32, tag=f"tp{s}", name=f"tp{s}")
            for c in range(4):
                cw = NJ * R // 4
                nc.sync.dma_start(TP[s][:, c * cw:(c + 1) * cw],
                                  tpack[s][:, c * cw:(c + 1) * cw])
        ID = main.tile([128, 128], F32, tag="id", name="id")
        nc.sync.dma_start(ID[:], ident[:, :])
        WA = main.tile([RA, S], F32, tag="wa", name="wa")
        nc.sync.dma_start(WA[:], wrep[0:RA, :])
        WB = main.tile([RB, S], F32, tag="wb", name="wb")
        nc.sync.dma_start(WB[:], wrep[RA:R, :])
        W1 = main.tile([128, S], F32, tag="w1", name="w1")
        nc.sync.dma_start(W1[:], w128[:, :])
        TA = main.tile([RA, D], F32, tag="ta", name="ta")
        nc.sync.dma_start(TA[:], tnat[0:RA, :])
        TB = main.tile([RB, D], F32, tag="tb", name="tb")
        nc.sync.dma_start(TB[:], tnat[RA:R, :])

        # PE warm-up: HAM clock-gates the PE to 1.2GHz until ~3.4us of
        # sustained activity. Chew tiny matmuls on memset constants while the
        # DMAs stream so the Gram matmuls run at full 2.4GHz.
        for w in range(64):
            wps = ups.tile([1, 1], F32, tag="u", name="warm")
            nc.tensor.matmul(wps[:], ccol_a[:], ccol_a[:, 0:1], start=True, stop=True)

        # Gram matrices. Ga = G[0:128, :] by matmul; Gb = G[128:150, :] from
        # symmetry: Gb[:, 0:128] = (Ga[:, 128:150])^T (PE transpose) and the
        # [22,22] corner by cheap N=22 matmuls.
        Ga, Gb = {}, {}
        for s in range(NSLOT):
            gpa = gps.tile([RA, R], F32, tag="g", name="g")
            for j in range(NJ):
                base = j * R
                nc.tensor.matmul(gpa[:], TP[s][:, base:base + RA], TP[s][:, base:base + R],
                                 start=(j == 0), stop=(j == NJ - 1))
            Ga[s] = main.tile([RA, R], F32, tag=f"ga{s}", name=f"ga{s}")
            nc.scalar.copy(Ga[s][:], gpa[:])
            gpc = gps.tile([RB, RB], F32, tag="g", name="gc")
            for j in range(NJ):
                base = j * R
                nc.tensor.matmul(gpc[:], TP[s][:, base + RA:base + R],
                                 TP[s][:, base + RA:base + R],
                                 start=(j == 0), stop=(j == NJ - 1))
            gtr = ups.tile([RB, RA], F32, tag="t", name="gtr", bufs=2)
            nc.tensor.transpose(gtr[:], Ga[s][:, RA:R], ID[:, :])
            Gb[s] = main.tile([RB, R], F32, tag=f"gb{s}", name=f"gb{s}")
            nc.scalar.copy(Gb[s][:, 0:RA], gtr[:])
            nc.scalar.copy(Gb[s][:, RA:R], gpc[:])

        beta = {}

        # ---- iteration 1: gamma is uniform 1/2304 (softmax of zeros) ----
        for s in range(NSLOT):
            u1 = ups.tile([1, R], F32, tag="u", name="u")
            nc.tensor.matmul(u1[:], ccol_a[:], Ga[s][:], start=True, stop=False)
            nc.tensor.matmul(u1[:], ccol_b[:], Gb[s][:], start=False, stop=True)
            usum = small.tile([1, 1], F32, tag=f"usum{s}", name=f"usum{s}")
            nc.vector.reduce_sum(usum[:], u1[:], axis=mybir.AxisListType.X)
            sq1 = small.tile([1, 1], F32, tag=f"sq1_{s}", name=f"sq1_{s}")
            nc.vector.tensor_scalar_mul(sq1[:], usum[:], GAMMA1)
            scale1 = _scale_chain(nc, small, sq1, f"s1_{s}")
            beta[s] = small.tile([1, R], F32, tag=f"beta1_{s}", name=f"beta1_{s}")
            nc.vector.tensor_scalar_mul(beta[s][:], u1[:], scale1[0:1, 0:1])

        # ---- iteration 2 ----
        for s in range(NSLOT):
            g2 = _softmax_rows(nc, small, beta[s], f"g2_{s}")
            g2a, g2b = _transpose_row(nc, small, ups, g2, RA, RB, ones, f"g2_{s}")
            u2 = ups.tile([1, R], F32, tag="u", name="u")
            nc.tensor.matmul(u2[:], g2a[:], Ga[s][:], start=True, stop=False)
            nc.tensor.matmul(u2[:], g2b[:], Gb[s][:], start=False, stop=True)
            ttr_out = small.tile([1, R], F32, tag=f"ttr2_{s}", name=f"ttr2_{s}")
            sq2 = small.tile([1, 1], F32, tag=f"sq2_{s}", name=f"sq2_{s}")
            nc.vector.tensor_mul(ttr_out[:], g2[:], u2[:])
            nc.vector.reduce_sum(sq2[:], ttr_out[:], axis=mybir.AxisListType.X)
            scale2 = _scale_chain(nc, small, sq2, f"s2_{s}")
            du = small.tile([1, R], F32, tag=f"du{s}", name=f"du{s}")
            nc.vector.tensor_scalar_mul(du[:], u2[:], scale2[0:1, 0:1])
            b2 = small.tile([1, R], F32, tag=f"beta2_{s}", name=f"beta2_{s}")
            nc.vector.tensor_add(b2[:], beta[s][:], du[:])
            beta[s] = b2

        # ---- iteration 3 ----
        g3d, scale3d, g3cols = {}, {}, {}
        for s in range(NSLOT):
            g3 = _softmax_rows(nc, small, beta[s], f"g3_{s}")
            g3a, g3b = _transpose_row(nc, small, ups, g3, RA, RB, ones, f"g3_{s}")
            u3 = ups.tile([1, R], F32, tag="u", name="u")
            nc.tensor.matmul(u3[:], g3a[:], Ga[s][:], start=True, stop=False)
            nc.tensor.matmul(u3[:], g3b[:], Gb[s][:], start=False, stop=True)
            ttr_out = small.tile([1, R], F32, tag=f"ttr3_{s}", name=f"ttr3_{s}")
            sq3 = small.tile([1, 1], F32, tag=f"sq3_{s}", name=f"sq3_{s}")
            nc.vector.tensor_mul(ttr_out[:], g3[:], u3[:])
            nc.vector.reduce_sum(sq3[:], ttr_out[:], axis=mybir.AxisListType.X)
            scale3d[s] = _scale_chain(nc, small, sq3, f"s3_{s}")
            g3d[s] = g3
            g3cols[s] = (g3a, g3b)

        # ---- final projection, slot 0 on PE:  out0 = C^T @ T_nat ----
        g3a, g3b = g3cols[0]
        scale3 = scale3d[0]
        sb = ups.tile([RA, 2], F32, tag="sb", name="sb")
        nc.tensor.matmul(sb[0:RA, 0:1], ones[0:1, 0:RA], scale3[0:1, 0:1], start=True, stop=True)
        nc.tensor.matmul(sb[0:RB, 1:2], ones[0:1, 0:RB], scale3[0:1, 0:1], start=True, stop=True)
        gta = small.tile([RA, 1], F32, tag="gta", name="gta")
        nc.vector.tensor_mul(gta[:], g3a[:], sb[0:RA, 0:1])
        gtb = small.tile([RB, 1], F32, tag="gtb", name="gtb")
        nc.vector.tensor_mul(gtb[:], g3b[:], sb[0:RB, 1:2])
        CA = small.tile([RA, S], F32, tag="cca", name="cca")
        nc.vector.tensor_scalar_mul(CA[:], WA[:], gta[0:RA, 0:1])
        CB = small.tile([RB, S], F32, tag="ccb", name="ccb")
        nc.vector.tensor_scalar_mul(CB[:], WB[:], gtb[0:RB, 0:1])

        # ---- final projection, slot 1 on DVE/ACT (PE runs slot 0 meanwhile):
        # sigma-cols = segreduce(T^T * bcast(gamma3*scale3)); out1^T tiles =
        # per-partition-scalar outer products
        g31 = g3d[1]
        gt1 = small.tile([1, R], F32, tag="gt1", name="gt1")
        nc.vector.tensor_scalar_mul(gt1[:], g31[:], scale3d[1][0:1, 0:1])
        gbc = ups.tile([RA, R], F32, tag="t", name="gbc", bufs=2)
        nc.tensor.matmul(gbc[:], ones[0:1, 0:RA], gt1[:], start=True, stop=True)
        gsb = main.tile([RA, R], F32, tag="gsb", name="gsb")
        nc.scalar.copy(gsb[:], gbc[:])

        tmp1 = main.tile([128, NJ * R], F32, tag="tmp1", name="tmp1")
        t3 = TP[1].rearrange("p (j r) -> p j r", r=R)
        o3 = tmp1.rearrange("p (j r) -> p j r", r=R)
        scols = main.tile([128, NJ], F32, tag="scols", name="scols")
        h = NJ // 2
        g3t = bass.AP(gsb.tensor, gsb.offset, [list(gsb.ap[0]), [0, h], [1, R]])
        for half in range(2):
            nc.vector.tensor_mul(o3[:, half * h:(half + 1) * h, :],
                                 t3[:, half * h:(half + 1) * h, :], g3t)
            nc.vector.reduce_sum(scols[:, half * h:(half + 1) * h],
                                 o3[:, half * h:(half + 1) * h, :],
                                 axis=mybir.AxisListType.X)

        # slot0 PE rounds (ACT stages psum->SBUF so DVE stays free for slot1)
        for rnd in range(3):
            ot = ops.tile([S, 1024], F32, tag="o", name="o")
            for n in range(2):
                c0 = rnd * 1024 + n * 512
                nc.tensor.matmul(ot[:, n * 512:(n + 1) * 512], CA[:], TA[:, c0:c0 + 512],
                                 start=True, stop=False)
                nc.tensor.matmul(ot[:, n * 512:(n + 1) * 512], CB[:], TB[:, c0:c0 + 512],
                                 start=False, stop=True)
            osb = main.tile([S, 1024], F32, tag=f"osb{rnd}", name=f"osb{rnd}")
            nc.scalar.copy(osb[:], ot[:])
            nc.sync.dma_start(out0[:, rnd * 1024:(rnd + 1) * 1024], osb[:])

        # slot1 outer products -> out1^T [128, NJ*S]
        o1t = main.tile([128, NJ * S], F32, tag="o1t", name="o1t")
        for j in range(NJ):
            nc.vector.tensor_scalar_mul(o1t[:, j * S:(j + 1) * S], W1[:], scols[0:128, j:j + 1])
        nc.sync.dma_start(out1[:, :], o1t[:])

    nc.compile()
    return nc


def get_program():
    if "nc" not in _PROGRAM_CACHE:
        _PROGRAM_CACHE["nc"] = build_program()
    return _PROGRAM_CACHE["nc"]


def _pack_inputs(visual, acoustic, va, W):
    """Build the per-core input maps (host-side sharding)."""
    visual = np.ascontiguousarray(visual, np.float32)
    acoustic = np.ascontiguousarray(acoustic, np.float32)
    va = np.ascontiguousarray(va, np.float32)
    W = np.ascontiguousarray(W, np.float32)
    eye = np.eye(128, dtype=np.float32)

    in_maps = []
    for core in range(N_CORES):
        tpack = np.zeros((NSLOT, 128, NJ * R), np.float32)
        tnat = np.zeros((R, D), np.float32)
        wrep = np.zeros((R, S), np.float32)
        w128 = np.zeros((128, S), np.float32)
        for slot, b in enumerate(ASSIGN[core]):
            if b is None:
                continue
            T = np.concatenate([visual[b], acoustic[b], va[b]], axis=0)  # [150, D]
            tpack[slot] = T.reshape(R, NJ, 128).transpose(2, 1, 0).reshape(128, NJ * R)
            ws = W[b].sum(axis=1)
            if slot == 0:
                tnat[:] = T
                wrep[:] = np.broadcast_to(ws[None, :], (R, S))
            else:
                w128[:] = np.broadcast_to(ws[None, :], (128, S))
        in_maps.append({"tpack": tpack, "tnat": tnat, "wrep": wrep,
                        "w128": w128, "ident": eye})
    return in_maps


def kernel(visual, acoustic, va, W, **run_kwargs):
    nc = get_program()
    in_maps = _pack_inputs(visual, acoustic, va, W)
    res = run_bass_kernel_spmd(nc, in_maps, list(range(N_CORES)), **run_kwargs)
    out = np.zeros((B, S, D), np.float32)
    for core in range(N_CORES):
        b0, b1 = ASSIGN[core]
        if b0 is not None:
            out[b0] = res.results[core]["out0"]
        if b1 is not None:
            # out1[p, j*S+i] = out[b1][i, 128j+p]
            arr = res.results[core]["out1"].reshape(128, NJ, S)
            out[b1] = arr.transpose(2, 1, 0).reshape(S, D)
    kernel.last_results = res
    return out
